# revision 12
# baseline (speedup 1.0000x reference)
"""GCN encoder (2-layer GCNConv) on 8 Trainium2 NeuronCores.

Strategy (dest-sharded graph parallel, bulk dma_gather):
  - Destinations sharded by node range across 8 cores (12500 each); within a
    core dests are sorted by in-degree descending and edges laid out in
    ELL-prefix "rounds": round t holds the t-th incoming edge of every dest
    with degree > t, so the scatter-add is a contiguous DVE add into the
    prefix of the accumulator.
  - The per-edge gather uses the Pool engine's bulk InstDMAGatherAnt ucode
    (one call covers ~12k edges; ~2-3 ns/edge descriptor emission) instead of
    per-128-row indirect DMAs (~1.1 us each).  dma_gather indices are int16,
    so the 100352-row u table is addressed as 25088 quad rows of 4 nodes
    (256B); each gathered quad is reduced to the wanted 16-float subrow with
    4 one-hot mask multiplies on DVE (masks precomputed on host).
  - Both layers aggregate 16-wide features from a replicated (AllGather'ed)
    u table in DRAM; layer 2 aggregates BEFORE the 16x64 transform.

out = D^-1/2 (A+I) D^-1/2 relu(D^-1/2 (A+I) D^-1/2 X W1 + b1) W2 + b2
with u = h * dinv:  s[c] = sum_{e: col=c} u[row_e] + u[c];  out_h = s * dinv + b
"""

import math
import sys

import numpy as np

if "/opt/trn_rl_repo" not in sys.path:
    sys.path.insert(0, "/opt/trn_rl_repo")

import concourse.bacc as bacc
import concourse.bass as bass
import concourse.mybir as mybir
import concourse.tile as tile
from concourse import bass_utils
from concourse.masks import make_identity

# ---------------------------------------------------------------- constants
N = 100000
E = 3200000
IN_C, HID, OUT_C = 128, 16, 64
NCORES = 8
SHARD = N // NCORES            # 12500 real dests per core
P = 128
QCH = (SHARD + P - 1) // P     # 98 column-chunks of 128 ranks
SHARD_PAD = QCH * P            # 12544
SFREE = QCH * HID              # 1568 f32 per partition for s
TABLE_ROWS = NCORES * SHARD_PAD  # 100352 rows in the replicated u table
QROWS = TABLE_ROWS // 4        # 25088 quad rows (4 nodes x 64B = 256B each)
PAD_DEG = 1.0e30               # huge degree for pad ranks -> dinv ~ 1e-15
GCOLS = 120                    # 128-edge columns per dma_gather call
NI_MAX = GCOLS * P             # 15360 idxs/call -> 961 ring descs (cap 1024)
NIW_MAX = NI_MAX // 16
SORT_SRC = False               # sorting by source row gave no HBM-locality win


def _set_gcols(g):
    global GCOLS, NI_MAX, NIW_MAX
    GCOLS, NI_MAX, NIW_MAX = g, g * P, g * P // 16
XCH = 8                        # xT streaming chunk (column-chunks of 128)

F32 = mybir.dt.float32
I16 = mybir.dt.int16

assert SHARD_PAD % 4 == 0  # quads never straddle a core's table region


def _round_profile():
    """Static per-round widths W_t (in 128-rank columns), from the Poisson(32)
    in-degree profile of E uniform edges over N nodes, with an 8-sigma + 64
    margin so any same-distribution input fits. Returns list of W_t."""
    lam = E / N
    R_MAX = 200
    pmf = np.zeros(R_MAX)
    pmf[0] = math.exp(-lam)
    for k in range(1, R_MAX):
        pmf[k] = pmf[k - 1] * lam / k
    sf = 1.0 - np.cumsum(pmf)  # sf[t] = P(X > t)
    W = []
    for t in range(R_MAX):
        q = max(sf[t], 0.0)
        if N * q < 1e-10 and t > lam:
            break
        nt = SHARD * q
        sig = math.sqrt(max(SHARD * q * (1.0 - q), 0.0))
        w = int(math.ceil((nt + 7.0 * sig + 32.0) / P))
        W.append(max(1, min(QCH, w)))
    W += [1] * 8
    return W


ROUND_W = _round_profile()  # static fallback; exact profile computed per input
OFFS_W = sum(ROUND_W)


def _make_calls(round_w):
    """Split the offs_w gather columns into dma_gather calls of <= GCOLS
    columns. Returns list of (ncols, pieces) where pieces = list of
    (local_col, width, sacc_col_offset)."""
    cum = np.cumsum([0] + list(round_w))
    offs_w = int(cum[-1])
    calls = []
    for c0 in range(0, offs_w, GCOLS):
        nc_ = min(GCOLS, offs_w - c0)
        pieces = []
        for t, w in enumerate(round_w):
            a, b = max(int(cum[t]), c0), min(int(cum[t + 1]), c0 + nc_)
            if a < b:
                pieces.append((a - c0, b - a, a - int(cum[t])))
        calls.append((nc_, pieces))
    return calls


# ---------------------------------------------------------------- device code
def _build_program(round_w=None):
    if round_w is None:
        round_w = ROUND_W
    calls = _make_calls(round_w)
    ncalls = len(calls)
    nc = bacc.Bacc(
        "TRN2",
        target_bir_lowering=False,
        debug=False,
        num_devices=NCORES,
        enable_partition_id=False,
        num_swdge_queues=4,
    )
    xT = nc.dram_tensor("xT", [P, SHARD_PAD], F32, kind="ExternalInput")
    deg_in = nc.dram_tensor("deg", [P, QCH], F32, kind="ExternalInput")
    idxs_in = nc.dram_tensor("idxs", [P, ncalls * NIW_MAX], I16,
                             kind="ExternalInput")
    msk_in = nc.dram_tensor("msk", [P, ncalls * 4 * GCOLS], F32,
                            kind="ExternalInput")
    w1_in = nc.dram_tensor("W1", [IN_C, HID], F32, kind="ExternalInput")
    w2_in = nc.dram_tensor("W2", [HID, OUT_C], F32, kind="ExternalInput")
    b1_in = nc.dram_tensor("b1", [P, HID], F32, kind="ExternalInput")
    b2_in = nc.dram_tensor("b2", [P, OUT_C], F32, kind="ExternalInput")
    out_d = nc.dram_tensor("out", [SHARD_PAD, OUT_C], F32, kind="ExternalOutput")

    with tile.TileContext(nc) as tc:
        with (
            tc.tile_pool(name="const", bufs=1) as cpool,
            tc.tile_pool(name="xstr", bufs=2) as xpool,
            tc.tile_pool(name="gath", bufs=2) as gpool,
            tc.tile_pool(name="gsel", bufs=2) as spool,
            tc.tile_pool(name="gtmp", bufs=2) as tpool,
            tc.tile_pool(name="idx", bufs=3) as ipool,
            tc.tile_pool(name="msk", bufs=3) as mpool,
            tc.tile_pool(name="psum", bufs=3, space="PSUM") as ppool,
            tc.tile_pool(name="psumT", bufs=3, space="PSUM") as ptpool,
            tc.tile_pool(name="dram", bufs=1, space="DRAM") as dpool,
        ):
            # ---- constants
            w1_sb = cpool.tile([IN_C, HID], F32, name="w1_sb")
            w2_sb = cpool.tile([HID, OUT_C], F32, name="w2_sb")
            b1_sb = cpool.tile([P, HID], F32, name="b1_sb")
            b2_sb = cpool.tile([P, OUT_C], F32, name="b2_sb")
            ident = cpool.tile([P, P], F32, name="ident")
            deg_sb = cpool.tile([P, QCH], F32, name="deg_sb")
            dinv = cpool.tile([P, QCH], F32, name="dinv")
            u_own = cpool.tile([P, SFREE], F32, name="u_own")
            u2_own = cpool.tile([P, SFREE], F32, name="u2_own")
            s_acc = cpool.tile([P, SFREE], F32, name="s_acc")
            v_sb = cpool.tile([P, SFREE], F32, name="v_sb")
            out_sb = cpool.tile([P, QCH * OUT_C], F32, name="out_sb")

            nc.sync.dma_start(out=w1_sb[:], in_=w1_in[:])
            nc.sync.dma_start(out=w2_sb[:], in_=w2_in[:])
            nc.sync.dma_start(out=b1_sb[:], in_=b1_in[:])
            nc.sync.dma_start(out=b2_sb[:], in_=b2_in[:])
            nc.sync.dma_start(out=deg_sb[:], in_=deg_in[:])
            make_identity(nc, ident[:])

            nc.vector.reciprocal(dinv[:], deg_sb[:])
            nc.scalar.activation(dinv[:], dinv[:], mybir.ActivationFunctionType.Sqrt)

            def dinv16():
                a = dinv[:]
                return bass.AP(a.tensor, a.offset, [a.ap[0], a.ap[1], [0, HID]])

            dram_u1own = dpool.tile([SHARD_PAD, HID], F32, name="dram_u1own")
            dram_u2own = dpool.tile([SHARD_PAD, HID], F32, name="dram_u2own")
            u1_tab = dpool.tile([TABLE_ROWS, HID], F32, name="u1_tab",
                                addr_space="Shared")
            u2_tab = dpool.tile([TABLE_ROWS, HID], F32, name="u2_tab",
                                addr_space="Shared")

            # ---- layer-1 transform: u1 = (x @ W1) * dinv (xT streamed,
            # matmuls grouped 32 chunks per PSUM bank, one DVE scale per bank)
            BNK = 32
            xt = None
            pg = None
            for q in range(QCH):
                if q % XCH == 0:
                    qn = min(XCH, QCH - q)
                    xt = xpool.tile([P, XCH * P], F32, name="xt", tag="xt")
                    nc.sync.dma_start(
                        out=xt[:, : qn * P],
                        in_=xT[:, q * P : (q + qn) * P],
                    )
                if q % BNK == 0:
                    pg = ppool.tile([P, BNK * HID], F32, name="mm1", tag="mm")
                nc.tensor.matmul(
                    out=pg[:, (q % BNK) * HID : (q % BNK + 1) * HID],
                    lhsT=xt[:, (q % XCH) * P : (q % XCH + 1) * P],
                    rhs=w1_sb[:],
                    start=True,
                    stop=True,
                )
                if (q + 1) % BNK == 0 or q == QCH - 1:
                    bn = q % BNK + 1
                    b0 = q - q % BNK
                    dv = dinv[:, b0 : b0 + bn]
                    dvb = bass.AP(dv.tensor, dv.offset,
                                  [dv.ap[0], dv.ap[1], [0, HID]])
                    nc.vector.tensor_tensor(
                        out=u_own[:, b0 * HID : (b0 + bn) * HID].rearrange(
                            "p (q f) -> p q f", f=HID),
                        in0=pg[:, : bn * HID].rearrange("p (q f) -> p q f",
                                                        f=HID),
                        in1=dvb,
                        op=mybir.AluOpType.mult,
                    )

            nc.sync.dma_start(
                out=dram_u1own[:].rearrange("(p q) f -> p (q f)", p=P),
                in_=u_own[:],
            )
            nc.gpsimd.collective_compute(
                "AllGather",
                mybir.AluOpType.bypass,
                replica_groups=[list(range(NCORES))],
                ins=[dram_u1own.opt()],
                outs=[u1_tab.opt()],
            )

            def quad_view(tab):
                a = tab[:]
                return bass.AP(a.tensor, a.offset, [[64, QROWS], [1, 64]])

            # ---- edge aggregation for one layer (bulk quad gather + select)
            def aggregate(tab, sacc, tag):
                nc.vector.memset(sacc[:], 0.0)
                tq = quad_view(tab)
                for k, (ncols, pieces) in enumerate(calls):
                    ni = ncols * P
                    it = ipool.tile([P, NIW_MAX], I16, name="it", tag="it")
                    nc.sync.dma_start(
                        out=it[:, : ni // 16],
                        in_=idxs_in[:, k * NIW_MAX : k * NIW_MAX + ni // 16],
                    )
                    mt = mpool.tile([P, 4 * GCOLS], F32, name="mt", tag="mt")
                    nc.sync.dma_start(
                        out=mt[:],
                        in_=msk_in[:, k * 4 * GCOLS : (k + 1) * 4 * GCOLS],
                    )
                    g = gpool.tile([P, GCOLS * 64], F32, name="g", tag="g")
                    nc.gpsimd.dma_gather(
                        g[:, : ncols * 64].rearrange("p (c e) -> p c e", e=64),
                        tq,
                        it[:, : ni // 16],
                        ni,
                        ni,
                        64,
                        single_packet=False,
                        queue_num=k % 4,
                    )
                    # one-hot select of the wanted 16-float subrow per column
                    gs = spool.tile([P, GCOLS * HID], F32, name="gs", tag="gs")
                    ga = g[:]
                    for cls in range(4):
                        gcls = bass.AP(
                            ga.tensor, ga.offset + cls * HID,
                            [ga.ap[0], [64, ncols], [1, HID]],
                        )
                        mm = mt[:, cls * GCOLS : cls * GCOLS + ncols]
                        mb = bass.AP(mm.tensor, mm.offset,
                                     [mm.ap[0], mm.ap[1], [0, HID]])
                        if cls == 0:
                            nc.vector.tensor_tensor(
                                out=gs[:, : ncols * HID].rearrange(
                                    "p (c f) -> p c f", f=HID),
                                in0=gcls, in1=mb, op=mybir.AluOpType.mult,
                            )
                        else:
                            tt = tpool.tile([P, GCOLS * HID], F32, name="tt",
                                            tag="tt")
                            nc.vector.tensor_tensor(
                                out=tt[:, : ncols * HID].rearrange(
                                    "p (c f) -> p c f", f=HID),
                                in0=gcls, in1=mb, op=mybir.AluOpType.mult,
                            )
                            nc.vector.tensor_tensor(
                                out=gs[:, : ncols * HID],
                                in0=gs[:, : ncols * HID],
                                in1=tt[:, : ncols * HID],
                                op=mybir.AluOpType.add,
                            )
                    for lc, w, soff in pieces:
                        nc.vector.tensor_tensor(
                            out=sacc[:, soff * HID : (soff + w) * HID],
                            in0=sacc[:, soff * HID : (soff + w) * HID],
                            in1=gs[:, lc * HID : (lc + w) * HID],
                            op=mybir.AluOpType.add,
                        )

            aggregate(u1_tab, s_acc, "l1")

            # self loop + finalize: u2 = relu((s + u1) * dinv + b1) * dinv
            def shaped(t):
                a = t[:]
                return a.rearrange("p (q f) -> p q f", f=HID)

            def b16(t, f):
                a = t[:]
                return bass.AP(a.tensor, a.offset, [a.ap[0], [0, QCH], [1, f]])

            nc.vector.tensor_tensor(
                out=s_acc[:], in0=s_acc[:], in1=u_own[:], op=mybir.AluOpType.add
            )
            nc.vector.tensor_tensor(
                out=shaped(s_acc), in0=shaped(s_acc), in1=dinv16(),
                op=mybir.AluOpType.mult,
            )
            nc.vector.tensor_tensor(
                out=shaped(s_acc), in0=shaped(s_acc), in1=b16(b1_sb, HID),
                op=mybir.AluOpType.add,
            )
            nc.scalar.activation(
                s_acc[:], s_acc[:], mybir.ActivationFunctionType.Relu
            )
            nc.vector.tensor_tensor(
                out=shaped(u2_own), in0=shaped(s_acc), in1=dinv16(),
                op=mybir.AluOpType.mult,
            )

            nc.sync.dma_start(
                out=dram_u2own[:].rearrange("(p q) f -> p (q f)", p=P),
                in_=u2_own[:],
            )
            nc.gpsimd.collective_compute(
                "AllGather",
                mybir.AluOpType.bypass,
                replica_groups=[list(range(NCORES))],
                ins=[dram_u2own.opt()],
                outs=[u2_tab.opt()],
            )

            # ---- layer-2 aggregation into v, then out = (v*dinv) @ W2 + b2
            aggregate(u2_tab, v_sb, "l2")
            nc.vector.tensor_tensor(
                out=v_sb[:], in0=v_sb[:], in1=u2_own[:], op=mybir.AluOpType.add
            )
            nc.vector.tensor_tensor(
                out=shaped(v_sb), in0=shaped(v_sb), in1=dinv16(),
                op=mybir.AluOpType.mult,
            )

            # head: matmuls grouped 8 chunks per PSUM bank; one fused
            # (+b2 broadcast) DVE copy per bank
            HG = 8
            pg2 = None
            for q in range(QCH):
                ptt = ptpool.tile([HID, P], F32, name="vT_ps", tag="vT_ps")
                nc.tensor.transpose(
                    out=ptt[:],
                    in_=v_sb[:, q * HID : (q + 1) * HID],
                    identity=ident[:],
                )
                vT = xpool.tile([HID, P], F32, name="vT_sb", tag="vT_sb")
                nc.vector.tensor_copy(out=vT[:], in_=ptt[:])
                if q % HG == 0:
                    pg2 = ppool.tile([P, HG * OUT_C], F32, name="mm2", tag="mm")
                nc.tensor.matmul(
                    out=pg2[:, (q % HG) * OUT_C : (q % HG + 1) * OUT_C],
                    lhsT=vT[:], rhs=w2_sb[:], start=True, stop=True,
                )
                if (q + 1) % HG == 0 or q == QCH - 1:
                    gn = q % HG + 1
                    g0 = q - q % HG
                    bb = b2_sb[:]
                    b2b = bass.AP(bb.tensor, bb.offset,
                                  [bb.ap[0], [0, gn], [1, OUT_C]])
                    nc.vector.tensor_tensor(
                        out=out_sb[:, g0 * OUT_C : (g0 + gn) * OUT_C].rearrange(
                            "p (q f) -> p q f", f=OUT_C),
                        in0=pg2[:, : gn * OUT_C].rearrange("p (q f) -> p q f",
                                                           f=OUT_C),
                        in1=b2b,
                        op=mybir.AluOpType.add,
                    )

            nc.sync.dma_start(
                out=out_d[:].rearrange("(p q) f -> p (q f)", p=P),
                in_=out_sb[:],
            )

    nc.compile()
    return nc


_NC_CACHE = {}


def _get_program(round_w=None):
    key = tuple(round_w) if round_w is not None else None
    if key not in _NC_CACHE:
        _NC_CACHE[key] = _build_program(round_w)
    return _NC_CACHE[key]


# ---------------------------------------------------------------- host prep
def _prep_inputs(x, edge_index, W1, b1, W2, b2):
    """Pure index preprocessing + layout (sharding). Returns in_maps and the
    inverse row permutation for unsharding."""
    x = np.asarray(x, dtype=np.float32)
    row = np.asarray(edge_index[0], dtype=np.int64)
    col = np.asarray(edge_index[1], dtype=np.int64)
    W1 = np.asarray(W1, dtype=np.float32)
    W2 = np.asarray(W2, dtype=np.float32)
    b1 = np.asarray(b1, dtype=np.float32).reshape(-1)
    b2 = np.asarray(b2, dtype=np.float32).reshape(-1)

    indeg = np.bincount(col, minlength=N).astype(np.int64)  # excl self loop
    deg = (indeg + 1).astype(np.float32)

    # per-core rank of each node: sort own range by in-degree descending
    rank = np.empty(N, dtype=np.int64)
    node_of_rank = np.empty((NCORES, SHARD_PAD), dtype=np.int64)
    for c in range(NCORES):
        nodes = np.arange(c * SHARD, (c + 1) * SHARD)
        order = np.argsort(-indeg[nodes], kind="stable")
        rank[nodes[order]] = np.arange(SHARD)
        node_of_rank[c, :SHARD] = nodes[order]
        node_of_rank[c, SHARD:] = -1

    # global u-table row of a node: core*SHARD_PAD + (rank%128)*QCH + rank//128
    core_of = np.arange(N) // SHARD
    table_row = core_of * SHARD_PAD + (rank % P) * QCH + rank // P

    # ELL-prefix round assignment: order edges by (core, dest rank, src row).
    # Sorting each dest's edges by source table row makes round t gather the
    # t-th order statistic of every dest's sources, so one round's accesses
    # cluster in a narrow band of the table (HBM page locality).
    dcore = col // SHARD
    drank = rank[col]
    ekey = dcore * SHARD_PAD + drank
    if SORT_SRC:
        eorder = np.lexsort((table_row[row], ekey))
    else:
        eorder = np.argsort(ekey, kind="stable")
    ekey_s = ekey[eorder]
    row_s = row[eorder]
    starts = np.searchsorted(ekey_s, np.arange(NCORES * SHARD_PAD))
    t_of = np.arange(E) - starts[ekey_s]

    # exact round profile for THIS input
    maxdeg = int(indeg.max())
    round_w = []
    for t in range(maxdeg):
        wt = 0
        for c in range(NCORES):
            n_tc = int(np.count_nonzero(indeg[c * SHARD:(c + 1) * SHARD] > t))
            wt = max(wt, (n_tc + P - 1) // P)
        round_w.append(max(1, wt))
    cum_w = np.cumsum([0] + round_w)
    offs_w = int(cum_w[-1])

    # slot arrays [NCORES, P, offs_w]: source table row per (partition, col)
    tr = np.zeros((NCORES, P, offs_w), dtype=np.int64)
    valid = np.zeros((NCORES, P, offs_w), dtype=bool)
    dr = ekey_s % SHARD_PAD
    dc = ekey_s // SHARD_PAD
    qq, pp = dr // P, dr % P
    wt = np.asarray(round_w + [0], dtype=np.int64)
    tcl = np.minimum(t_of, len(round_w) - 1)
    ok = (t_of < len(round_w)) & (qq < wt[tcl])
    if not np.all(ok):
        raise RuntimeError("round profile exceeded: slot overflow")
    cidx = cum_w[t_of] + qq
    tr[dc, pp, cidx] = table_row[row_s]
    valid[dc, pp, cidx] = True
    idxq = (tr // 4).astype(np.int16)       # quad row (pads -> 0, masked off)
    sel = (tr % 4).astype(np.int8)

    # per-call int16 idx wrap + one-hot masks
    ncalls = (offs_w + GCOLS - 1) // GCOLS
    idxs_all = np.zeros((NCORES, P, ncalls * NIW_MAX), dtype=np.int16)
    msk_all = np.zeros((NCORES, P, ncalls * 4 * GCOLS), dtype=np.float32)
    for k in range(ncalls):
        c0 = k * GCOLS
        nc_ = min(GCOLS, offs_w - c0)
        ni = nc_ * P
        blk = idxq[:, :, c0:c0 + nc_]                     # [NCORES, P, nc_]
        lin = blk.transpose(0, 2, 1).reshape(NCORES, ni)  # j = c*128 + p
        w = lin.reshape(NCORES, ni // 16, 16).transpose(0, 2, 1)
        idxs_all[:, :, k * NIW_MAX: k * NIW_MAX + ni // 16] = np.tile(
            w, (1, P // 16, 1))
        sblk = sel[:, :, c0:c0 + nc_]
        vblk = valid[:, :, c0:c0 + nc_]
        base = k * 4 * GCOLS
        for cls in range(4):
            msk_all[:, :, base + cls * GCOLS: base + cls * GCOLS + nc_] = (
                (sblk == cls) & vblk)

    # per-core tensors
    in_maps = []
    b1b = np.broadcast_to(b1, (P, HID)).astype(np.float32).copy()
    b2b = np.broadcast_to(b2, (P, OUT_C)).astype(np.float32).copy()
    for c in range(NCORES):
        nor = node_of_rank[c]
        deg_pi = np.full(SHARD_PAD, PAD_DEG, dtype=np.float32)
        deg_pi[:SHARD] = deg[nor[:SHARD]]
        deg_sb = deg_pi.reshape(QCH, P).T.copy()
        xT = np.zeros((P, SHARD_PAD), dtype=np.float32)
        xT[:, :SHARD] = x[nor[:SHARD]].T
        in_maps.append(
            {
                "xT": np.ascontiguousarray(xT),
                "deg": np.ascontiguousarray(deg_sb),
                "idxs": np.ascontiguousarray(idxs_all[c]),
                "msk": np.ascontiguousarray(msk_all[c]),
                "W1": W1,
                "W2": W2,
                "b1": b1b,
                "b2": b2b,
            }
        )

    inv_rows = table_row
    global OFFS_W
    OFFS_W = offs_w
    return in_maps, inv_rows, round_w


def _build_floor_probe():
    """Minimal 8-core program for measuring the PJRT dispatch floor."""
    nc = bacc.Bacc("TRN2", target_bir_lowering=False, debug=False,
                   num_devices=NCORES, enable_partition_id=False)
    a = nc.dram_tensor("a", [P, 16], F32, kind="ExternalInput")
    b = nc.dram_tensor("b", [P, 16], F32, kind="ExternalOutput")
    with tile.TileContext(nc) as tc:
        with tc.tile_pool(name="sb", bufs=1) as sb:
            t = sb.tile([P, 16], F32, name="t")
            nc.sync.dma_start(out=t[:], in_=a[:])
            nc.sync.dma_start(out=b[:], in_=t[:])
    nc.compile()
    return nc


def timed_run(in_maps, reps=5, nc=None, round_w=None):
    """Time device execution of the compiled program (PJRT path, inputs
    pre-staged on device). Returns best wall-ns per execution."""
    import time

    import jax
    from jax.sharding import Mesh, PartitionSpec
    from jax.experimental.shard_map import shard_map as _shard_map

    if nc is None:
        nc = _get_program(round_w)
    import concourse.mybir as _mb
    from concourse.bass2jax import _bass_exec_p, install_neuronx_cc_hook

    install_neuronx_cc_hook()
    in_names, out_names, out_avals, zero_outs = [], [], [], []
    for alloc in nc.m.functions[0].allocations:
        if not isinstance(alloc, _mb.MemoryLocationSet):
            continue
        name = alloc.memorylocations[0].name
        if alloc.kind == "ExternalInput":
            in_names.append(name)
        elif alloc.kind == "ExternalOutput":
            out_names.append(name)
            shape = tuple(alloc.tensor_shape)
            dtype = _mb.dt.np(alloc.dtype)
            out_avals.append(jax.core.ShapedArray(shape, dtype))
            zero_outs.append(np.zeros(shape, dtype))
    n_params = len(in_names)
    all_in_names = in_names + out_names

    def _body(*args):
        return tuple(
            _bass_exec_p.bind(
                *args,
                out_avals=tuple(out_avals),
                in_names=tuple(all_in_names),
                out_names=tuple(out_names),
                lowering_input_output_aliases=(),
                sim_require_finite=True,
                sim_require_nnan=True,
                nc=nc,
            )
        )

    devices = jax.devices()[:NCORES]
    mesh = Mesh(np.asarray(devices), ("core",))
    nio = n_params + len(out_names)
    fn = jax.jit(
        _shard_map(
            _body,
            mesh=mesh,
            in_specs=(PartitionSpec("core"),) * nio,
            out_specs=(PartitionSpec("core"),) * len(out_names),
            check_rep=False,
        )
    )
    concat_in = [
        np.concatenate([np.asarray(in_maps[c][nm]) for c in range(NCORES)], axis=0)
        for nm in in_names
    ] + [np.concatenate([z] * NCORES, axis=0) for z in zero_outs]
    sharding = jax.sharding.NamedSharding(mesh, PartitionSpec("core"))
    handles = [jax.device_put(a, sharding) for a in concat_in]
    best = None
    for _ in range(reps):
        t0 = time.perf_counter()
        outs = fn(*handles)
        jax.block_until_ready(outs)
        dt = time.perf_counter() - t0
        if best is None or dt < best:
            best = dt
    return best * 1e9


def kernel(x, edge_index, W1, b1, W2, b2):
    in_maps, inv_rows, round_w = _prep_inputs(x, edge_index, W1, b1, W2, b2)
    nc = _get_program(round_w)
    res = bass_utils.run_bass_kernel_spmd(
        nc, in_maps, core_ids=list(range(NCORES))
    )
    outs = np.concatenate(
        [res.results[c]["out"] for c in range(NCORES)], axis=0
    )  # [NCORES*SHARD_PAD, OUT_C]
    return np.ascontiguousarray(outs[inv_rows]).astype(np.float32)
